# revision 14
# baseline (speedup 1.0000x reference)
"""Batch K-Means (VQ codebook EMA update) on 8 TRN2 NeuronCores.

Strategy: data-parallel over N (32768 rows -> 4096 per core), codebook
replicated. Each core computes, for its row shard:
  - scores[n,k] = Xn[n,:] @ C[k,:]^T - 0.5*|c_k|^2   (fp32 matmul; argmax
    of score == argmin of distance; bf16 flips ~15/32768 indices so the
    score matmul must be fp32)
  - idx[n] = argmax_k scores (DVE max8 + max_index, first-occurrence ties)
  - dw_partial[k,d] = sum_{n: idx[n]=k} X[n,d]  (one-hot blocks regenerated
    from idx on DVE in fp16, contracted on the PE in fp16)
Host does the cheap O(K*D) tail: all-reduce of dw partials, bincount of
indices, EMA update, and the quantized gather.
"""

import numpy as np
import ml_dtypes

from concourse import bacc, mybir
import concourse.bass as bass
import concourse.tile as tile
from concourse.bass_utils import run_bass_kernel_spmd

N_CLUSTERS = 2048
EMBED_DIM = 512
DECAY = 0.99
EPSILON = 1e-05
NORM_EPS = 1e-12

N_CORES = 8
N_TOTAL = 32768
R = N_TOTAL // N_CORES          # rows per core = 4096
P = 128                         # partitions
RC = R // P                     # row chunks per core = 32
DC = EMBED_DIM // P             # contraction chunks = 4
KC = N_CLUSTERS // 512          # score psum chunks = 4
KH = 2                          # dw k-halves
KO = N_CLUSTERS // 2 // P       # dw k-chunks per half = 8

f32 = mybir.dt.float32
f16 = mybir.dt.float16
bf16 = mybir.dt.bfloat16
u32 = mybir.dt.uint32


RES_SCALE = 64.0  # X-residual scaling keeps fp16 operands out of denormals


def build_nc():
    nc = bacc.Bacc("TRN2", target_bir_lowering=False, debug=False,
                   num_devices=N_CORES)
    # Scores run as fp16 hi/lo two-matmul decomposition:
    #   score = Xh @ Ct + (64*Xl) @ (Ct/64),  Xh=fp16(Xn), Xl=fp16(Xn-Xh)
    # validated exact-index vs fp32 on the fixed inputs (0/32768 flips,
    # min top-2 margin 5e-4 >> device rounding noise), at 2x the speed
    # of the fp32 matmul path (which runs as 2 half-rate passes).
    xnt_d = nc.dram_tensor("xnt", [EMBED_DIM, R], f16, kind="ExternalInput")
    xlt_d = nc.dram_tensor("xlt", [EMBED_DIM, R], f16, kind="ExternalInput")
    xb_d = nc.dram_tensor("xb", [R, EMBED_DIM], f16, kind="ExternalInput")
    ct_d = nc.dram_tensor("ct", [EMBED_DIM, N_CLUSTERS], f16,
                          kind="ExternalInput")
    cts_d = nc.dram_tensor("cts", [EMBED_DIM, N_CLUSTERS], f16,
                           kind="ExternalInput")
    c2h_d = nc.dram_tensor("c2h", [P, N_CLUSTERS], f32, kind="ExternalInput")
    io16_d = nc.dram_tensor("io16", [P, N_CLUSTERS], f16,
                            kind="ExternalInput")
    idx_d = nc.dram_tensor("idx", [R], f32, kind="ExternalOutput")
    dw_d = nc.dram_tensor("dw", [N_CLUSTERS, EMBED_DIM], f32,
                          kind="ExternalOutput")

    with tile.TileContext(nc) as tc:
        with (
            tc.tile_pool(name="const", bufs=1) as const,
            tc.tile_pool(name="score", bufs=3) as spool,
            tc.tile_pool(name="small", bufs=4) as small,
            tc.tile_pool(name="oh", bufs=4) as ohpool,
            tc.tile_pool(name="ev", bufs=2) as evpool,
        ):
            xnt_sb = const.tile([P, DC, R], f16)
            xlt_sb = const.tile([P, DC, R], f16)
            ct_sb = const.tile([P, DC, N_CLUSTERS], f16)
            cts_sb = const.tile([P, DC, N_CLUSTERS], f16)
            xb_sb = const.tile([P, RC, EMBED_DIM], f16)
            c2h_sb = const.tile([P, N_CLUSTERS], f32)
            io16_sb = const.tile([P, N_CLUSTERS], f16)
            idxf_sb = const.tile([P, RC], f32)

            # DMA order: score operands first (critical path to matmul 0),
            # split in n-pieces so chunk-0 starts early; phase-B-only loads
            # (xb, io16) go on the gpsimd queue to keep sync dispatch short.
            # One dma_start = one DMA queue, so split every load per-dc and
            # issue in critical-path order: ct + xnt piece0 feed matmul 0,
            # then cts + xlt piece0 (lo-half of chunk 0), then the rest.
            for dc in range(DC):
                nc.sync.dma_start(ct_sb[:, dc, :],
                                  ct_d[dc * P:(dc + 1) * P, :])
            NPIECE = 4
            W = R // NPIECE
            for dc in range(DC):
                nc.sync.dma_start(xnt_sb[:, dc, 0:W], xnt_d[dc * P:(dc + 1) * P, 0:W])
            for dc in range(DC):
                nc.sync.dma_start(cts_sb[:, dc, :],
                                  cts_d[dc * P:(dc + 1) * P, :])
            for dc in range(DC):
                nc.sync.dma_start(xlt_sb[:, dc, 0:W], xlt_d[dc * P:(dc + 1) * P, 0:W])
            nc.sync.dma_start(c2h_sb[:], c2h_d[:])
            for pc in range(1, NPIECE):
                n0, n1 = pc * W, (pc + 1) * W
                for dc in range(DC):
                    nc.sync.dma_start(xnt_sb[:, dc, n0:n1],
                                      xnt_d[dc * P:(dc + 1) * P, n0:n1])
                    nc.sync.dma_start(xlt_sb[:, dc, n0:n1],
                                      xlt_d[dc * P:(dc + 1) * P, n0:n1])
            nc.gpsimd.dma_start(io16_sb[:], io16_d[:])
            nc.gpsimd.dma_start(
                xb_sb[:], xb_d[:].rearrange("(i p) d -> p i d", p=P))

            # ---- Phase A: scores + argmax per row chunk ----
            with tc.tile_pool(name="psA", bufs=8,
                              space=bass.MemorySpace.PSUM) as psA:
                for i in range(RC):
                    score = spool.tile([P, N_CLUSTERS], f32, tag="score")
                    for kc in range(KC):
                        s = psA.tile([P, 512], f32, tag="ps")
                        for dc in range(DC):
                            nc.tensor.matmul(
                                s[:],
                                xnt_sb[:, dc, i * P:(i + 1) * P],
                                ct_sb[:, dc, kc * 512:(kc + 1) * 512],
                                start=(dc == 0), stop=False)
                        for dc in range(DC):
                            nc.tensor.matmul(
                                s[:],
                                xlt_sb[:, dc, i * P:(i + 1) * P],
                                cts_sb[:, dc, kc * 512:(kc + 1) * 512],
                                start=False, stop=(dc == DC - 1))
                        nc.vector.tensor_tensor(
                            out=score[:, kc * 512:(kc + 1) * 512],
                            in0=s[:],
                            in1=c2h_sb[:, kc * 512:(kc + 1) * 512],
                            op=mybir.AluOpType.subtract)
                    m8 = small.tile([P, 8], f32, tag="m8")
                    i8 = small.tile([P, 8], u32, tag="i8")
                    nc.vector.max(m8[:], score[:])
                    nc.vector.max_index(i8[:], m8[:], score[:])
                    nc.vector.tensor_copy(idxf_sb[:, i:i + 1], i8[:, 0:1])

            nc.sync.dma_start(idx_d[:].rearrange("(i p) -> p i", p=P),
                              idxf_sb[:])

            # ---- Phase B: dw = onehot^T @ X, 4 k-quarter passes so one
            # quarter's evacuation overlaps the next quarter's matmuls ----
            KQ = 4
            KOQ = KO * KH // KQ  # 4 psum banks per quarter
            with tc.tile_pool(name="psB", bufs=2,
                              space=bass.MemorySpace.PSUM) as psB:
                for q in range(KQ):
                    ps = [psB.tile([P, EMBED_DIM], f32, tag=f"dw{ko}",
                                   name=f"psdw_{q}_{ko}")
                          for ko in range(KOQ)]
                    for i in range(RC):
                        oh = ohpool.tile([P, KOQ * P], f16, tag="oh")
                        nc.vector.tensor_scalar(
                            out=oh[:],
                            in0=io16_sb[:, q * KOQ * P:(q + 1) * KOQ * P],
                            scalar1=idxf_sb[:, i:i + 1],
                            scalar2=None,
                            op0=mybir.AluOpType.is_equal)
                        for ko in range(KOQ):
                            nc.tensor.matmul(
                                ps[ko][:],
                                oh[:, ko * P:(ko + 1) * P],
                                xb_sb[:, i, :],
                                start=(i == 0), stop=(i == RC - 1))
                    for ko in range(KOQ):
                        ev = evpool.tile([P, EMBED_DIM], f32, tag="ev")
                        nc.scalar.copy(ev[:], ps[ko][:])
                        k0 = (q * KOQ + ko) * P
                        nc.sync.dma_start(dw_d[k0:k0 + P, :], ev[:])

    nc.compile()
    return nc


_NC_CACHE = None


def _get_nc():
    global _NC_CACHE
    if _NC_CACHE is None:
        _NC_CACHE = build_nc()
    return _NC_CACHE


def make_in_maps(X, centroids):
    norms = np.linalg.norm(X, axis=1, keepdims=True)
    Xn = X / np.maximum(norms, NORM_EPS)
    Xh = Xn.astype(np.float16)
    Xl = ((Xn - Xh.astype(np.float32)) * RES_SCALE).astype(np.float16)
    XhT = np.ascontiguousarray(Xh.T)                       # [512, 32768]
    XlT = np.ascontiguousarray(Xl.T)
    CT = np.ascontiguousarray(centroids.T.astype(np.float16))
    CTs = np.ascontiguousarray(
        (centroids.T / RES_SCALE).astype(np.float16))
    c2h = 0.5 * (centroids * centroids).sum(axis=1)        # [2048]
    c2h_b = np.ascontiguousarray(
        np.broadcast_to(c2h[None, :], (P, N_CLUSTERS))).astype(np.float32)
    io16 = np.ascontiguousarray(np.broadcast_to(
        np.arange(N_CLUSTERS, dtype=np.float16)[None, :], (P, N_CLUSTERS)))
    xb16 = X.astype(np.float16)
    in_maps = []
    for c in range(N_CORES):
        sl = slice(c * R, (c + 1) * R)
        in_maps.append({
            "xnt": np.ascontiguousarray(XhT[:, sl]),
            "xlt": np.ascontiguousarray(XlT[:, sl]),
            "xb": np.ascontiguousarray(xb16[sl]),
            "ct": CT,
            "cts": CTs,
            "c2h": c2h_b,
            "io16": io16,
        })
    return in_maps


def postprocess(X, centroids, ema_cluster_size, ema_w, idx_full, dw):
    counts = np.bincount(idx_full, minlength=N_CLUSTERS).astype(np.float32)
    quantized = centroids[idx_full]
    new_size = ema_cluster_size * DECAY + (1.0 - DECAY) * counts
    n = new_size.sum(dtype=np.float32)
    new_size = (new_size + EPSILON) / (n + N_CLUSTERS * EPSILON) * n
    new_w = ema_w * DECAY + (1.0 - DECAY) * dw
    new_centroids = new_w / new_size[:, None]
    return (quantized, idx_full[:, None].astype(np.int32), new_centroids,
            new_size, new_w)


def kernel(X, centroids, ema_cluster_size, ema_w):
    X = np.asarray(X, dtype=np.float32)
    centroids = np.asarray(centroids, dtype=np.float32)
    ema_cluster_size = np.asarray(ema_cluster_size, dtype=np.float32)
    ema_w = np.asarray(ema_w, dtype=np.float32)

    nc = _get_nc()
    in_maps = make_in_maps(X, centroids)
    res = None
    last_exc = None
    for attempt in range(3):
        try:
            res = run_bass_kernel_spmd(nc, in_maps, list(range(N_CORES)))
            break
        except Exception as e:  # transient device errors: reset + retry
            last_exc = e
            try:
                import ctypes
                lib = ctypes.CDLL('/opt/axon/libaxon_pjrt.so')
                lib.axon_reset.restype = ctypes.c_int64
                lib.axon_reset()
            except Exception:
                pass
            import time
            time.sleep(20 * (attempt + 1))
    if res is None:
        raise last_exc

    idx_full = np.concatenate(
        [res.results[c]["idx"] for c in range(N_CORES)]).astype(np.int32)
    dw = np.zeros((N_CLUSTERS, EMBED_DIM), dtype=np.float32)
    for c in range(N_CORES):
        dw += res.results[c]["dw"]
    return postprocess(X, centroids, ema_cluster_size, ema_w, idx_full, dw)


# revision 18
# speedup vs baseline: 1.0121x; 1.0121x over previous
"""Batch K-Means (VQ codebook EMA update) on 8 TRN2 NeuronCores.

Strategy: data-parallel over N (32768 rows -> 4096 per core), codebook
replicated. Each core computes, for its row shard:
  - scores[n,k] = Xn[n,:] @ C[k,:]^T - 0.5*|c_k|^2   (fp32 matmul; argmax
    of score == argmin of distance; bf16 flips ~15/32768 indices so the
    score matmul must be fp32)
  - idx[n] = argmax_k scores (DVE max8 + max_index, first-occurrence ties)
  - dw_partial[k,d] = sum_{n: idx[n]=k} X[n,d]  (one-hot blocks regenerated
    from idx on DVE in fp16, contracted on the PE in fp16)
Host does the cheap O(K*D) tail: all-reduce of dw partials, bincount of
indices, EMA update, and the quantized gather.
"""

import numpy as np
import ml_dtypes

from concourse import bacc, mybir
import concourse.bass as bass
import concourse.tile as tile
from concourse.bass_utils import run_bass_kernel_spmd

N_CLUSTERS = 2048
EMBED_DIM = 512
DECAY = 0.99
EPSILON = 1e-05
NORM_EPS = 1e-12

N_CORES = 8
N_TOTAL = 32768
R = N_TOTAL // N_CORES          # rows per core = 4096
P = 128                         # partitions
RC = R // P                     # row chunks per core = 32
DC = EMBED_DIM // P             # contraction chunks = 4
KC = N_CLUSTERS // 512          # score psum chunks = 4
KH = 2                          # dw k-halves
KO = N_CLUSTERS // 2 // P       # dw k-chunks per half = 8

f32 = mybir.dt.float32
f16 = mybir.dt.float16
bf16 = mybir.dt.bfloat16
u32 = mybir.dt.uint32


RES_SCALE = 64.0  # X-residual scaling keeps fp16 operands out of denormals


def build_nc():
    nc = bacc.Bacc("TRN2", target_bir_lowering=False, debug=False,
                   num_devices=N_CORES)
    # Scores run as fp16 hi/lo two-matmul decomposition:
    #   score = Xh @ Ct + (64*Xl) @ (Ct/64),  Xh=fp16(Xn), Xl=fp16(Xn-Xh)
    # validated exact-index vs fp32 on the fixed inputs (0/32768 flips,
    # min top-2 margin 5e-4 >> device rounding noise), at 2x the speed
    # of the fp32 matmul path (which runs as 2 half-rate passes).
    xnt_d = nc.dram_tensor("xnt", [EMBED_DIM, R], f16, kind="ExternalInput")
    xlt_d = nc.dram_tensor("xlt", [EMBED_DIM, R], f16, kind="ExternalInput")
    xb_d = nc.dram_tensor("xb", [R, EMBED_DIM], f16, kind="ExternalInput")
    ct_d = nc.dram_tensor("ct", [EMBED_DIM, N_CLUSTERS], f16,
                          kind="ExternalInput")
    cts_d = nc.dram_tensor("cts", [EMBED_DIM, N_CLUSTERS], f16,
                           kind="ExternalInput")
    c2h_d = nc.dram_tensor("c2h", [P, N_CLUSTERS], f32, kind="ExternalInput")
    io16_d = nc.dram_tensor("io16", [P, N_CLUSTERS], f16,
                            kind="ExternalInput")
    idx_d = nc.dram_tensor("idx", [R], f32, kind="ExternalOutput")
    dw_d = nc.dram_tensor("dw", [N_CLUSTERS, EMBED_DIM], f32,
                          kind="ExternalOutput")

    with tile.TileContext(nc) as tc:
        with (
            tc.tile_pool(name="const", bufs=1) as const,
            tc.tile_pool(name="score", bufs=3) as spool,
            tc.tile_pool(name="small", bufs=4) as small,
            tc.tile_pool(name="oh", bufs=4) as ohpool,
            tc.tile_pool(name="ev", bufs=2) as evpool,
        ):
            xnt_sb = const.tile([P, DC, R], f16)
            xlt_sb = const.tile([P, DC, R], f16)
            ct_sb = const.tile([P, DC, N_CLUSTERS], f16)
            cts_sb = const.tile([P, DC, N_CLUSTERS], f16)
            xb_sb = const.tile([P, RC, EMBED_DIM], f16)
            c2h_sb = const.tile([P, N_CLUSTERS], f32)
            io16_sb = const.tile([P, N_CLUSTERS], f16)
            idxf_sb = const.tile([P, RC], f32)

            # DMA order: score operands first (critical path to matmul 0),
            # split in n-pieces so chunk-0 starts early; phase-B-only loads
            # (xb, io16) go on the gpsimd queue to keep sync dispatch short.
            # One dma_start = one DMA queue, so split every load per-dc and
            # issue in critical-path order: ct + xnt piece0 feed matmul 0,
            # then cts + xlt piece0 (lo-half of chunk 0), then the rest.
            for dc in range(DC):
                nc.sync.dma_start(ct_sb[:, dc, :],
                                  ct_d[dc * P:(dc + 1) * P, :])
            NPIECE = 4
            W = R // NPIECE
            for dc in range(DC):
                nc.sync.dma_start(xnt_sb[:, dc, 0:W], xnt_d[dc * P:(dc + 1) * P, 0:W])
            for dc in range(DC):
                nc.sync.dma_start(cts_sb[:, dc, :],
                                  cts_d[dc * P:(dc + 1) * P, :])
            for dc in range(DC):
                nc.sync.dma_start(xlt_sb[:, dc, 0:W], xlt_d[dc * P:(dc + 1) * P, 0:W])
            nc.sync.dma_start(c2h_sb[:], c2h_d[:])
            for pc in range(1, NPIECE):
                n0, n1 = pc * W, (pc + 1) * W
                for dc in range(DC):
                    nc.sync.dma_start(xnt_sb[:, dc, n0:n1],
                                      xnt_d[dc * P:(dc + 1) * P, n0:n1])
                    nc.sync.dma_start(xlt_sb[:, dc, n0:n1],
                                      xlt_d[dc * P:(dc + 1) * P, n0:n1])
            nc.gpsimd.dma_start(io16_sb[:], io16_d[:])
            nc.gpsimd.dma_start(
                xb_sb[:], xb_d[:].rearrange("(i p) d -> p i d", p=P))

            # ---- Phase A: scores + argmax per row chunk ----
            with tc.tile_pool(name="psA", bufs=2,
                              space=bass.MemorySpace.PSUM) as psA:
                for i in range(RC):
                    score = spool.tile([P, N_CLUSTERS], f32, tag="score")
                    sw = psA.tile([P, N_CLUSTERS], f32, tag="ps")
                    for kc in range(KC):
                        s = sw[:, kc * 512:(kc + 1) * 512]
                        for dc in range(DC):
                            nc.tensor.matmul(
                                s,
                                xnt_sb[:, dc, i * P:(i + 1) * P],
                                ct_sb[:, dc, kc * 512:(kc + 1) * 512],
                                start=(dc == 0), stop=False)
                        for dc in range(DC):
                            nc.tensor.matmul(
                                s,
                                xlt_sb[:, dc, i * P:(i + 1) * P],
                                cts_sb[:, dc, kc * 512:(kc + 1) * 512],
                                start=False, stop=(dc == DC - 1))
                    nc.vector.tensor_tensor(
                        out=score[:], in0=sw[:], in1=c2h_sb[:],
                        op=mybir.AluOpType.subtract)
                    m8 = small.tile([P, 8], f32, tag="m8")
                    i8 = small.tile([P, 8], u32, tag="i8")
                    nc.vector.max(m8[:], score[:])
                    nc.vector.max_index(i8[:], m8[:], score[:])
                    nc.vector.tensor_copy(idxf_sb[:, i:i + 1], i8[:, 0:1])

            nc.sync.dma_start(idx_d[:].rearrange("(i p) -> p i", p=P),
                              idxf_sb[:])

            # ---- Phase B: dw = onehot^T @ X, 4 k-quarter passes so one
            # quarter's evacuation overlaps the next quarter's matmuls ----
            KQ = 4
            KOQ = KO * KH // KQ  # 4 psum banks per quarter
            with tc.tile_pool(name="psB", bufs=2,
                              space=bass.MemorySpace.PSUM) as psB:
                for q in range(KQ):
                    ps = [psB.tile([P, EMBED_DIM], f32, tag=f"dw{ko}",
                                   name=f"psdw_{q}_{ko}")
                          for ko in range(KOQ)]
                    for i in range(RC):
                        oh = ohpool.tile([P, KOQ * P], f16, tag="oh")
                        nc.vector.tensor_scalar(
                            out=oh[:],
                            in0=io16_sb[:, q * KOQ * P:(q + 1) * KOQ * P],
                            scalar1=idxf_sb[:, i:i + 1],
                            scalar2=None,
                            op0=mybir.AluOpType.is_equal)
                        for ko in range(KOQ):
                            nc.tensor.matmul(
                                ps[ko][:],
                                oh[:, ko * P:(ko + 1) * P],
                                xb_sb[:, i, :],
                                start=(i == 0), stop=(i == RC - 1))
                    for ko in range(KOQ):
                        ev = evpool.tile([P, EMBED_DIM], f32, tag="ev")
                        nc.scalar.copy(ev[:], ps[ko][:])
                        k0 = (q * KOQ + ko) * P
                        nc.sync.dma_start(dw_d[k0:k0 + P, :], ev[:])

    nc.compile()
    return nc


_NC_CACHE = None


def _get_nc():
    global _NC_CACHE
    if _NC_CACHE is None:
        _NC_CACHE = build_nc()
    return _NC_CACHE


def make_in_maps(X, centroids):
    norms = np.linalg.norm(X, axis=1, keepdims=True)
    Xn = X / np.maximum(norms, NORM_EPS)
    Xh = Xn.astype(np.float16)
    Xl = ((Xn - Xh.astype(np.float32)) * RES_SCALE).astype(np.float16)
    XhT = np.ascontiguousarray(Xh.T)                       # [512, 32768]
    XlT = np.ascontiguousarray(Xl.T)
    CT = np.ascontiguousarray(centroids.T.astype(np.float16))
    CTs = np.ascontiguousarray(
        (centroids.T / RES_SCALE).astype(np.float16))
    c2h = 0.5 * (centroids * centroids).sum(axis=1)        # [2048]
    c2h_b = np.ascontiguousarray(
        np.broadcast_to(c2h[None, :], (P, N_CLUSTERS))).astype(np.float32)
    io16 = np.ascontiguousarray(np.broadcast_to(
        np.arange(N_CLUSTERS, dtype=np.float16)[None, :], (P, N_CLUSTERS)))
    xb16 = X.astype(np.float16)
    in_maps = []
    for c in range(N_CORES):
        sl = slice(c * R, (c + 1) * R)
        in_maps.append({
            "xnt": np.ascontiguousarray(XhT[:, sl]),
            "xlt": np.ascontiguousarray(XlT[:, sl]),
            "xb": np.ascontiguousarray(xb16[sl]),
            "ct": CT,
            "cts": CTs,
            "c2h": c2h_b,
            "io16": io16,
        })
    return in_maps


def postprocess(X, centroids, ema_cluster_size, ema_w, idx_full, dw):
    counts = np.bincount(idx_full, minlength=N_CLUSTERS).astype(np.float32)
    quantized = centroids[idx_full]
    new_size = ema_cluster_size * DECAY + (1.0 - DECAY) * counts
    n = new_size.sum(dtype=np.float32)
    new_size = (new_size + EPSILON) / (n + N_CLUSTERS * EPSILON) * n
    new_w = ema_w * DECAY + (1.0 - DECAY) * dw
    new_centroids = new_w / new_size[:, None]
    return (quantized, idx_full[:, None].astype(np.int32), new_centroids,
            new_size, new_w)


def kernel(X, centroids, ema_cluster_size, ema_w):
    X = np.asarray(X, dtype=np.float32)
    centroids = np.asarray(centroids, dtype=np.float32)
    ema_cluster_size = np.asarray(ema_cluster_size, dtype=np.float32)
    ema_w = np.asarray(ema_w, dtype=np.float32)

    nc = _get_nc()
    in_maps = make_in_maps(X, centroids)
    res = None
    last_exc = None
    for attempt in range(4):
        try:
            res = run_bass_kernel_spmd(nc, in_maps, list(range(N_CORES)))
            break
        except Exception as e:  # transient device errors: reset + retry
            last_exc = e
            try:
                import ctypes
                lib = ctypes.CDLL('/opt/axon/libaxon_pjrt.so')
                lib.axon_reset.restype = ctypes.c_int64
                lib.axon_reset()
            except Exception:
                pass
            import time
            time.sleep(30 * (attempt + 1))
    if res is None:
        raise last_exc

    idx_full = np.concatenate(
        [res.results[c]["idx"] for c in range(N_CORES)]).astype(np.int32)
    dw = np.zeros((N_CLUSTERS, EMBED_DIM), dtype=np.float32)
    for c in range(N_CORES):
        dw += res.results[c]["dw"]
    return postprocess(X, centroids, ema_cluster_size, ema_w, idx_full, dw)


# revision 19
# speedup vs baseline: 1.0554x; 1.0428x over previous
"""Batch K-Means (VQ codebook EMA update) on 8 TRN2 NeuronCores.

Strategy: data-parallel over N (32768 rows -> 4096 per core), codebook
replicated. Each core computes, for its row shard:
  - scores[n,k] = Xn[n,:] @ C[k,:]^T - 0.5*|c_k|^2   (fp32 matmul; argmax
    of score == argmin of distance; bf16 flips ~15/32768 indices so the
    score matmul must be fp32)
  - idx[n] = argmax_k scores (DVE max8 + max_index, first-occurrence ties)
  - dw_partial[k,d] = sum_{n: idx[n]=k} X[n,d]  (one-hot blocks regenerated
    from idx on DVE in fp16, contracted on the PE in fp16)
Host does the cheap O(K*D) tail: all-reduce of dw partials, bincount of
indices, EMA update, and the quantized gather.
"""

import numpy as np
import ml_dtypes

from concourse import bacc, mybir
import concourse.bass as bass
import concourse.tile as tile
from concourse.bass_utils import run_bass_kernel_spmd

N_CLUSTERS = 2048
EMBED_DIM = 512
DECAY = 0.99
EPSILON = 1e-05
NORM_EPS = 1e-12

N_CORES = 8
N_TOTAL = 32768
R = N_TOTAL // N_CORES          # rows per core = 4096
P = 128                         # partitions
RC = R // P                     # row chunks per core = 32
DC = EMBED_DIM // P             # contraction chunks = 4
KC = N_CLUSTERS // 512          # score psum chunks = 4
KH = 2                          # dw k-halves
KO = N_CLUSTERS // 2 // P       # dw k-chunks per half = 8

f32 = mybir.dt.float32
f16 = mybir.dt.float16
bf16 = mybir.dt.bfloat16
u32 = mybir.dt.uint32


RES_SCALE = 64.0  # X-residual scaling keeps fp16 operands out of denormals


def build_nc():
    nc = bacc.Bacc("TRN2", target_bir_lowering=False, debug=False,
                   num_devices=N_CORES)
    # Scores run as fp16 hi/lo two-matmul decomposition:
    #   score = Xh @ Ct + (64*Xl) @ (Ct/64),  Xh=fp16(Xn), Xl=fp16(Xn-Xh)
    # validated exact-index vs fp32 on the fixed inputs (0/32768 flips,
    # min top-2 margin 5e-4 >> device rounding noise), at 2x the speed
    # of the fp32 matmul path (which runs as 2 half-rate passes).
    xnt_d = nc.dram_tensor("xnt", [EMBED_DIM, R], f16, kind="ExternalInput")
    xlt_d = nc.dram_tensor("xlt", [EMBED_DIM, R], f16, kind="ExternalInput")
    xb_d = nc.dram_tensor("xb", [R, EMBED_DIM], f16, kind="ExternalInput")
    ct_d = nc.dram_tensor("ct", [EMBED_DIM, N_CLUSTERS], f16,
                          kind="ExternalInput")
    cts_d = nc.dram_tensor("cts", [EMBED_DIM, N_CLUSTERS], f16,
                           kind="ExternalInput")
    c2h_d = nc.dram_tensor("c2h", [P, N_CLUSTERS], f32, kind="ExternalInput")
    io16_d = nc.dram_tensor("io16", [P, N_CLUSTERS], f16,
                            kind="ExternalInput")
    idx_d = nc.dram_tensor("idx", [R], f32, kind="ExternalOutput")
    dw_d = nc.dram_tensor("dw", [N_CLUSTERS, EMBED_DIM], f32,
                          kind="ExternalOutput")

    with tile.TileContext(nc) as tc:
        with (
            tc.tile_pool(name="const", bufs=1) as const,
            tc.tile_pool(name="score", bufs=3) as spool,
            tc.tile_pool(name="small", bufs=4) as small,
            tc.tile_pool(name="oh", bufs=4) as ohpool,
            tc.tile_pool(name="ev", bufs=2) as evpool,
        ):
            xnt_sb = const.tile([P, DC, R], f16)
            xlt_sb = const.tile([P, DC, R], f16)
            ct_sb = const.tile([P, DC, N_CLUSTERS], f16)
            cts_sb = const.tile([P, DC, N_CLUSTERS], f16)
            xb_sb = const.tile([P, RC, EMBED_DIM], f16)
            c2h_sb = const.tile([P, N_CLUSTERS], f32)
            io16_sb = const.tile([P, N_CLUSTERS], f16)
            idxf_sb = const.tile([P, RC], f32)

            # DMA order: score operands first (critical path to matmul 0),
            # split in n-pieces so chunk-0 starts early; phase-B-only loads
            # (xb, io16) go on the gpsimd queue to keep sync dispatch short.
            # One dma_start = one DMA queue, so split every load per-dc and
            # issue in critical-path order: ct + xnt piece0 feed matmul 0,
            # then cts + xlt piece0 (lo-half of chunk 0), then the rest.
            for dc in range(DC):
                nc.sync.dma_start(ct_sb[:, dc, :],
                                  ct_d[dc * P:(dc + 1) * P, :])
            NPIECE = 4
            W = R // NPIECE
            for dc in range(DC):
                nc.sync.dma_start(xnt_sb[:, dc, 0:W], xnt_d[dc * P:(dc + 1) * P, 0:W])
            for dc in range(DC):
                nc.sync.dma_start(cts_sb[:, dc, :],
                                  cts_d[dc * P:(dc + 1) * P, :])
            for dc in range(DC):
                nc.sync.dma_start(xlt_sb[:, dc, 0:W], xlt_d[dc * P:(dc + 1) * P, 0:W])
            nc.sync.dma_start(c2h_sb[:], c2h_d[:])
            for pc in range(1, NPIECE):
                n0, n1 = pc * W, (pc + 1) * W
                for dc in range(DC):
                    nc.sync.dma_start(xnt_sb[:, dc, n0:n1],
                                      xnt_d[dc * P:(dc + 1) * P, n0:n1])
                    nc.sync.dma_start(xlt_sb[:, dc, n0:n1],
                                      xlt_d[dc * P:(dc + 1) * P, n0:n1])
            xb_r = xb_d[:].rearrange("(i p) d -> p i d", p=P)
            for pc in range(NPIECE):
                i0, i1 = pc * (RC // NPIECE), (pc + 1) * (RC // NPIECE)
                nc.sync.dma_start(xb_sb[:, i0:i1, :], xb_r[:, i0:i1, :])
            nc.sync.dma_start(io16_sb[:], io16_d[:])

            # ---- Phase A: scores + argmax per row chunk ----
            with tc.tile_pool(name="psA", bufs=2,
                              space=bass.MemorySpace.PSUM) as psA:
                for i in range(RC):
                    score = spool.tile([P, N_CLUSTERS], f32, tag="score")
                    sw = psA.tile([P, N_CLUSTERS], f32, tag="ps")
                    for kc in range(KC):
                        s = sw[:, kc * 512:(kc + 1) * 512]
                        for dc in range(DC):
                            nc.tensor.matmul(
                                s,
                                xnt_sb[:, dc, i * P:(i + 1) * P],
                                ct_sb[:, dc, kc * 512:(kc + 1) * 512],
                                start=(dc == 0), stop=False)
                        for dc in range(DC):
                            nc.tensor.matmul(
                                s,
                                xlt_sb[:, dc, i * P:(i + 1) * P],
                                cts_sb[:, dc, kc * 512:(kc + 1) * 512],
                                start=False, stop=(dc == DC - 1))
                    nc.vector.tensor_tensor(
                        out=score[:], in0=sw[:], in1=c2h_sb[:],
                        op=mybir.AluOpType.subtract)
                    m8 = small.tile([P, 8], f32, tag="m8")
                    i8 = small.tile([P, 8], u32, tag="i8")
                    nc.vector.max(m8[:], score[:])
                    nc.vector.max_index(i8[:], m8[:], score[:])
                    nc.vector.tensor_copy(idxf_sb[:, i:i + 1], i8[:, 0:1])

            nc.sync.dma_start(idx_d[:].rearrange("(i p) -> p i", p=P),
                              idxf_sb[:])

            # ---- Phase B: dw = onehot^T @ X, 4 k-quarter passes so one
            # quarter's evacuation overlaps the next quarter's matmuls ----
            KQ = 4
            KOQ = KO * KH // KQ  # 4 psum banks per quarter
            with tc.tile_pool(name="psB", bufs=2,
                              space=bass.MemorySpace.PSUM) as psB:
                for q in range(KQ):
                    ps = [psB.tile([P, EMBED_DIM], f32, tag=f"dw{ko}",
                                   name=f"psdw_{q}_{ko}")
                          for ko in range(KOQ)]
                    for i in range(RC):
                        oh = ohpool.tile([P, KOQ * P], f16, tag="oh")
                        nc.vector.tensor_scalar(
                            out=oh[:],
                            in0=io16_sb[:, q * KOQ * P:(q + 1) * KOQ * P],
                            scalar1=idxf_sb[:, i:i + 1],
                            scalar2=None,
                            op0=mybir.AluOpType.is_equal)
                        for ko in range(KOQ):
                            nc.tensor.matmul(
                                ps[ko][:],
                                oh[:, ko * P:(ko + 1) * P],
                                xb_sb[:, i, :],
                                start=(i == 0), stop=(i == RC - 1))
                    for ko in range(KOQ):
                        ev = evpool.tile([P, EMBED_DIM], f32, tag="ev")
                        nc.scalar.copy(ev[:], ps[ko][:])
                        k0 = (q * KOQ + ko) * P
                        nc.sync.dma_start(dw_d[k0:k0 + P, :], ev[:])

    nc.compile()
    return nc


_NC_CACHE = None


def _get_nc():
    global _NC_CACHE
    if _NC_CACHE is None:
        _NC_CACHE = build_nc()
    return _NC_CACHE


def make_in_maps(X, centroids):
    norms = np.linalg.norm(X, axis=1, keepdims=True)
    Xn = X / np.maximum(norms, NORM_EPS)
    Xh = Xn.astype(np.float16)
    Xl = ((Xn - Xh.astype(np.float32)) * RES_SCALE).astype(np.float16)
    XhT = np.ascontiguousarray(Xh.T)                       # [512, 32768]
    XlT = np.ascontiguousarray(Xl.T)
    CT = np.ascontiguousarray(centroids.T.astype(np.float16))
    CTs = np.ascontiguousarray(
        (centroids.T / RES_SCALE).astype(np.float16))
    c2h = 0.5 * (centroids * centroids).sum(axis=1)        # [2048]
    c2h_b = np.ascontiguousarray(
        np.broadcast_to(c2h[None, :], (P, N_CLUSTERS))).astype(np.float32)
    io16 = np.ascontiguousarray(np.broadcast_to(
        np.arange(N_CLUSTERS, dtype=np.float16)[None, :], (P, N_CLUSTERS)))
    xb16 = X.astype(np.float16)
    in_maps = []
    for c in range(N_CORES):
        sl = slice(c * R, (c + 1) * R)
        in_maps.append({
            "xnt": np.ascontiguousarray(XhT[:, sl]),
            "xlt": np.ascontiguousarray(XlT[:, sl]),
            "xb": np.ascontiguousarray(xb16[sl]),
            "ct": CT,
            "cts": CTs,
            "c2h": c2h_b,
            "io16": io16,
        })
    return in_maps


def postprocess(X, centroids, ema_cluster_size, ema_w, idx_full, dw):
    counts = np.bincount(idx_full, minlength=N_CLUSTERS).astype(np.float32)
    quantized = centroids[idx_full]
    new_size = ema_cluster_size * DECAY + (1.0 - DECAY) * counts
    n = new_size.sum(dtype=np.float32)
    new_size = (new_size + EPSILON) / (n + N_CLUSTERS * EPSILON) * n
    new_w = ema_w * DECAY + (1.0 - DECAY) * dw
    new_centroids = new_w / new_size[:, None]
    return (quantized, idx_full[:, None].astype(np.int32), new_centroids,
            new_size, new_w)


def kernel(X, centroids, ema_cluster_size, ema_w):
    X = np.asarray(X, dtype=np.float32)
    centroids = np.asarray(centroids, dtype=np.float32)
    ema_cluster_size = np.asarray(ema_cluster_size, dtype=np.float32)
    ema_w = np.asarray(ema_w, dtype=np.float32)

    nc = _get_nc()
    in_maps = make_in_maps(X, centroids)
    res = None
    last_exc = None
    for attempt in range(4):
        try:
            res = run_bass_kernel_spmd(nc, in_maps, list(range(N_CORES)))
            break
        except Exception as e:  # transient device errors: reset + retry
            last_exc = e
            try:
                import ctypes
                lib = ctypes.CDLL('/opt/axon/libaxon_pjrt.so')
                lib.axon_reset.restype = ctypes.c_int64
                lib.axon_reset()
            except Exception:
                pass
            import time
            time.sleep(30 * (attempt + 1))
    if res is None:
        raise last_exc

    idx_full = np.concatenate(
        [res.results[c]["idx"] for c in range(N_CORES)]).astype(np.int32)
    dw = np.zeros((N_CLUSTERS, EMBED_DIM), dtype=np.float32)
    for c in range(N_CORES):
        dw += res.results[c]["dw"]
    return postprocess(X, centroids, ema_cluster_size, ema_w, idx_full, dw)


# revision 22
# speedup vs baseline: 1.0592x; 1.0036x over previous
"""Batch K-Means (VQ codebook EMA update) on 8 TRN2 NeuronCores.

Strategy: data-parallel over N (32768 rows -> 4096 per core), codebook
replicated. Each core computes, for its row shard:
  - scores[n,k] = Xn[n,:] @ C[k,:]^T - 0.5*|c_k|^2  (argmax of score ==
    argmin of distance). The matmul runs as an fp16 hi/lo two-pass
    decomposition (see build_nc) because single-precision-truncated
    inputs flip argmax indices (bf16: 15/32768, fp16 single: 1/32768)
    while fp16 hi/lo matches the fp32 reference exactly at 2x the speed
    of the native fp32 matmul path.
  - idx[n] = argmax_k scores (DVE max8 + max_index, first-occurrence ties)
  - dw_partial[k,d] = sum_{n: idx[n]=k} X[n,d]  (one-hot blocks regenerated
    from idx on DVE in fp16, contracted on the PE in fp16)
Host does the cheap O(K*D) tail: all-reduce of dw partials, bincount of
indices, EMA update, and the quantized gather.
"""

import numpy as np

from concourse import bacc, mybir
import concourse.bass as bass
import concourse.tile as tile
from concourse.bass_utils import run_bass_kernel_spmd

N_CLUSTERS = 2048
EMBED_DIM = 512
DECAY = 0.99
EPSILON = 1e-05
NORM_EPS = 1e-12

N_CORES = 8
N_TOTAL = 32768
R = N_TOTAL // N_CORES          # rows per core = 4096
P = 128                         # partitions
RC = R // P                     # row chunks per core = 32
DC = EMBED_DIM // P             # contraction chunks = 4
KC = N_CLUSTERS // 512          # score psum chunks = 4
KH = 2                          # dw k-halves
KO = N_CLUSTERS // 2 // P       # dw k-chunks per half = 8

f32 = mybir.dt.float32
f16 = mybir.dt.float16
bf16 = mybir.dt.bfloat16
u32 = mybir.dt.uint32


RES_SCALE = 64.0  # X-residual scaling keeps fp16 operands out of denormals


def build_nc():
    nc = bacc.Bacc("TRN2", target_bir_lowering=False, debug=False,
                   num_devices=N_CORES)
    # Scores run as fp16 hi/lo two-matmul decomposition:
    #   score = Xh @ Ct + (64*Xl) @ (Ct/64),  Xh=fp16(Xn), Xl=fp16(Xn-Xh)
    # validated exact-index vs fp32 on the fixed inputs (0/32768 flips,
    # min top-2 margin 5e-4 >> device rounding noise), at 2x the speed
    # of the fp32 matmul path (which runs as 2 half-rate passes).
    xnt_d = nc.dram_tensor("xnt", [EMBED_DIM, R], f16, kind="ExternalInput")
    xlt_d = nc.dram_tensor("xlt", [EMBED_DIM, R], f16, kind="ExternalInput")
    xb_d = nc.dram_tensor("xb", [R, EMBED_DIM], f16, kind="ExternalInput")
    ct_d = nc.dram_tensor("ct", [EMBED_DIM, N_CLUSTERS], f16,
                          kind="ExternalInput")
    cts_d = nc.dram_tensor("cts", [EMBED_DIM, N_CLUSTERS], f16,
                           kind="ExternalInput")
    c2h_d = nc.dram_tensor("c2h", [P, N_CLUSTERS], f32, kind="ExternalInput")
    io16_d = nc.dram_tensor("io16", [P, N_CLUSTERS], f16,
                            kind="ExternalInput")
    idx_d = nc.dram_tensor("idx", [R], f32, kind="ExternalOutput")
    dw_d = nc.dram_tensor("dw", [N_CLUSTERS, EMBED_DIM], f32,
                          kind="ExternalOutput")

    with tile.TileContext(nc) as tc:
        with (
            tc.tile_pool(name="const", bufs=1) as const,
            tc.tile_pool(name="score", bufs=3) as spool,
            tc.tile_pool(name="small", bufs=4) as small,
            tc.tile_pool(name="oh", bufs=4) as ohpool,
            tc.tile_pool(name="ev", bufs=2) as evpool,
        ):
            xnt_sb = const.tile([P, DC, R], f16)
            xlt_sb = const.tile([P, DC, R], f16)
            ct_sb = const.tile([P, DC, N_CLUSTERS], f16)
            cts_sb = const.tile([P, DC, N_CLUSTERS], f16)
            xb_sb = const.tile([P, RC, EMBED_DIM], f16)
            c2h_sb = const.tile([P, N_CLUSTERS], f32)
            io16_sb = const.tile([P, N_CLUSTERS], f16)
            idxf_sb = const.tile([P, RC], f32)

            # One dma_start = one DMA queue, so split every load per-dc and
            # issue in critical-path order: ct + xnt piece0 feed matmul 0,
            # then cts + xlt piece0 (lo-half of chunk 0), then the rest;
            # phase-B-only loads (xb, io16) go last.
            for dc in range(DC):
                nc.sync.dma_start(ct_sb[:, dc, :],
                                  ct_d[dc * P:(dc + 1) * P, :])
            NPIECE = 4
            W = R // NPIECE
            for dc in range(DC):
                nc.sync.dma_start(xnt_sb[:, dc, 0:W], xnt_d[dc * P:(dc + 1) * P, 0:W])
            for dc in range(DC):
                nc.sync.dma_start(cts_sb[:, dc, :],
                                  cts_d[dc * P:(dc + 1) * P, :])
            for dc in range(DC):
                nc.sync.dma_start(xlt_sb[:, dc, 0:W], xlt_d[dc * P:(dc + 1) * P, 0:W])
            nc.sync.dma_start(c2h_sb[:], c2h_d[:])
            for pc in range(1, NPIECE):
                n0, n1 = pc * W, (pc + 1) * W
                for dc in range(DC):
                    nc.sync.dma_start(xnt_sb[:, dc, n0:n1],
                                      xnt_d[dc * P:(dc + 1) * P, n0:n1])
                    nc.sync.dma_start(xlt_sb[:, dc, n0:n1],
                                      xlt_d[dc * P:(dc + 1) * P, n0:n1])
            xb_r = xb_d[:].rearrange("(i p) d -> p i d", p=P)
            for pc in range(NPIECE):
                i0, i1 = pc * (RC // NPIECE), (pc + 1) * (RC // NPIECE)
                nc.sync.dma_start(xb_sb[:, i0:i1, :], xb_r[:, i0:i1, :])
            nc.sync.dma_start(io16_sb[:], io16_d[:])

            # ---- Phase A: scores + argmax per row chunk ----
            with tc.tile_pool(name="psA", bufs=2,
                              space=bass.MemorySpace.PSUM) as psA:
                for i in range(RC):
                    score = spool.tile([P, N_CLUSTERS], f32, tag="score")
                    sw = psA.tile([P, N_CLUSTERS], f32, tag="ps")
                    for kc in range(KC):
                        s = sw[:, kc * 512:(kc + 1) * 512]
                        for dc in range(DC):
                            nc.tensor.matmul(
                                s,
                                xnt_sb[:, dc, i * P:(i + 1) * P],
                                ct_sb[:, dc, kc * 512:(kc + 1) * 512],
                                start=(dc == 0), stop=False)
                        for dc in range(DC):
                            nc.tensor.matmul(
                                s,
                                xlt_sb[:, dc, i * P:(i + 1) * P],
                                cts_sb[:, dc, kc * 512:(kc + 1) * 512],
                                start=False, stop=(dc == DC - 1))
                    nc.vector.tensor_tensor(
                        out=score[:], in0=sw[:], in1=c2h_sb[:],
                        op=mybir.AluOpType.subtract)
                    m8 = small.tile([P, 8], f32, tag="m8")
                    i8 = small.tile([P, 8], u32, tag="i8")
                    nc.vector.max(m8[:], score[:])
                    nc.vector.max_index(i8[:], m8[:], score[:])
                    nc.vector.tensor_copy(idxf_sb[:, i:i + 1], i8[:, 0:1])

            nc.sync.dma_start(idx_d[:].rearrange("(i p) -> p i", p=P),
                              idxf_sb[:])

            # ---- Phase B: dw = onehot^T @ X, 4 k-quarter passes so one
            # quarter's evacuation overlaps the next quarter's matmuls ----
            KQ = 4
            KOQ = KO * KH // KQ  # 4 psum banks per quarter
            with tc.tile_pool(name="psB", bufs=2,
                              space=bass.MemorySpace.PSUM) as psB:
                for q in range(KQ):
                    ps = [psB.tile([P, EMBED_DIM], f32, tag=f"dw{ko}",
                                   name=f"psdw_{q}_{ko}")
                          for ko in range(KOQ)]
                    for i in range(RC):
                        oh = ohpool.tile([P, KOQ * P], f16, tag="oh")
                        nc.vector.tensor_scalar(
                            out=oh[:],
                            in0=io16_sb[:, q * KOQ * P:(q + 1) * KOQ * P],
                            scalar1=idxf_sb[:, i:i + 1],
                            scalar2=None,
                            op0=mybir.AluOpType.is_equal)
                        for ko in range(KOQ):
                            nc.tensor.matmul(
                                ps[ko][:],
                                oh[:, ko * P:(ko + 1) * P],
                                xb_sb[:, i, :],
                                start=(i == 0), stop=(i == RC - 1))
                    for ko in range(KOQ):
                        ev = evpool.tile([P, EMBED_DIM], f32, tag="ev")
                        nc.scalar.copy(ev[:], ps[ko][:])
                        k0 = (q * KOQ + ko) * P
                        nc.sync.dma_start(dw_d[k0:k0 + P, :], ev[:])

    nc.compile()
    return nc


_NC_CACHE = None


def _get_nc():
    global _NC_CACHE
    if _NC_CACHE is None:
        _NC_CACHE = build_nc()
    return _NC_CACHE


def make_in_maps(X, centroids):
    norms = np.linalg.norm(X, axis=1, keepdims=True)
    Xn = X / np.maximum(norms, NORM_EPS)
    Xh = Xn.astype(np.float16)
    Xl = ((Xn - Xh.astype(np.float32)) * RES_SCALE).astype(np.float16)
    XhT = np.ascontiguousarray(Xh.T)                       # [512, 32768]
    XlT = np.ascontiguousarray(Xl.T)
    CT = np.ascontiguousarray(centroids.T.astype(np.float16))
    CTs = np.ascontiguousarray(
        (centroids.T / RES_SCALE).astype(np.float16))
    c2h = 0.5 * (centroids * centroids).sum(axis=1)        # [2048]
    c2h_b = np.ascontiguousarray(
        np.broadcast_to(c2h[None, :], (P, N_CLUSTERS))).astype(np.float32)
    io16 = np.ascontiguousarray(np.broadcast_to(
        np.arange(N_CLUSTERS, dtype=np.float16)[None, :], (P, N_CLUSTERS)))
    xb16 = X.astype(np.float16)
    in_maps = []
    for c in range(N_CORES):
        sl = slice(c * R, (c + 1) * R)
        in_maps.append({
            "xnt": np.ascontiguousarray(XhT[:, sl]),
            "xlt": np.ascontiguousarray(XlT[:, sl]),
            "xb": np.ascontiguousarray(xb16[sl]),
            "ct": CT,
            "cts": CTs,
            "c2h": c2h_b,
            "io16": io16,
        })
    return in_maps


def postprocess(X, centroids, ema_cluster_size, ema_w, idx_full, dw):
    counts = np.bincount(idx_full, minlength=N_CLUSTERS).astype(np.float32)
    quantized = centroids[idx_full]
    new_size = ema_cluster_size * DECAY + (1.0 - DECAY) * counts
    n = new_size.sum(dtype=np.float32)
    new_size = (new_size + EPSILON) / (n + N_CLUSTERS * EPSILON) * n
    new_w = ema_w * DECAY + (1.0 - DECAY) * dw
    new_centroids = new_w / new_size[:, None]
    return (quantized, idx_full[:, None].astype(np.int32), new_centroids,
            new_size, new_w)


def kernel(X, centroids, ema_cluster_size, ema_w):
    X = np.asarray(X, dtype=np.float32)
    centroids = np.asarray(centroids, dtype=np.float32)
    ema_cluster_size = np.asarray(ema_cluster_size, dtype=np.float32)
    ema_w = np.asarray(ema_w, dtype=np.float32)

    nc = _get_nc()
    in_maps = make_in_maps(X, centroids)
    res = None
    last_exc = None
    for attempt in range(4):
        try:
            res = run_bass_kernel_spmd(nc, in_maps, list(range(N_CORES)))
            break
        except Exception as e:  # transient device errors: reset + retry
            last_exc = e
            try:
                import ctypes
                lib = ctypes.CDLL('/opt/axon/libaxon_pjrt.so')
                lib.axon_reset.restype = ctypes.c_int64
                lib.axon_reset()
            except Exception:
                pass
            import time
            time.sleep(30 * (attempt + 1))
    if res is None:
        raise last_exc

    idx_full = np.concatenate(
        [res.results[c]["idx"] for c in range(N_CORES)]).astype(np.int32)
    dw = np.zeros((N_CLUSTERS, EMBED_DIM), dtype=np.float32)
    for c in range(N_CORES):
        dw += res.results[c]["dw"]
    return postprocess(X, centroids, ema_cluster_size, ema_w, idx_full, dw)


# revision 28
# speedup vs baseline: 1.2614x; 1.1909x over previous
"""Batch K-Means (VQ codebook EMA update) on 8 TRN2 NeuronCores.

Strategy: data-parallel over N (32768 rows -> 4096 per core), codebook
replicated. Each core computes, for its row shard:
  - scores[n,k] = Xn[n,:] @ C[k,:]^T - 0.5*|c_k|^2  (argmax of score ==
    argmin of distance). The matmul runs with fp16 operands (fp32 PSUM
    accumulate); the -0.5|c|^2 bias is folded in as a contraction-2
    fp16 hi/lo rank update. fp16 rounding can flip ~1/32768 argmax
    indices, so the device also emits top-8 values+indices per row and
    the HOST rescores low-margin rows (top-2 gap < 8e-3, ~5 rows) in
    exact fp32 against their top-8 candidates, patching idx and dw.
  - idx[n] = argmax_k scores (DVE max8 + max_index, first-occurrence ties)
  - dw_partial[k,d] = sum_{n: idx[n]=k} X[n,d]  (one-hot blocks regenerated
    from idx on DVE in fp16, contracted on the PE in fp16)
Host does the cheap O(K*D) tail: all-reduce of dw partials, bincount of
indices, EMA update, and the quantized gather.
"""

import numpy as np

from concourse import bacc, mybir
import concourse.bass as bass
import concourse.tile as tile
from concourse.bass_utils import run_bass_kernel_spmd

N_CLUSTERS = 2048
EMBED_DIM = 512
DECAY = 0.99
EPSILON = 1e-05
NORM_EPS = 1e-12

N_CORES = 8
N_TOTAL = 32768
R = N_TOTAL // N_CORES          # rows per core = 4096
P = 128                         # partitions
RC = R // P                     # row chunks per core = 32
DC = EMBED_DIM // P             # contraction chunks = 4
KC = N_CLUSTERS // 512          # score psum chunks = 4
KH = 2                          # dw k-halves
KO = N_CLUSTERS // 2 // P       # dw k-chunks per half = 8

f32 = mybir.dt.float32
f16 = mybir.dt.float16
bf16 = mybir.dt.bfloat16
u32 = mybir.dt.uint32



def build_nc():
    nc = bacc.Bacc("TRN2", target_bir_lowering=False, debug=False,
                   num_devices=N_CORES)
    xnt_d = nc.dram_tensor("xnt", [EMBED_DIM, R], f16, kind="ExternalInput")
    xb_d = nc.dram_tensor("xb", [R, EMBED_DIM], f16, kind="ExternalInput")
    ct_d = nc.dram_tensor("ct", [EMBED_DIM, N_CLUSTERS], f16,
                          kind="ExternalInput")
    # -0.5|c|^2 bias folded into the matmul as a contraction-2 rank update:
    # lhsT = [[1..],[2^-11..]], rhs = [f16(bias); f16((bias-hi)*2048)]
    one2_d = nc.dram_tensor("one2", [2, P], f16, kind="ExternalInput")
    bias2_d = nc.dram_tensor("bias2", [2, N_CLUSTERS], f16,
                             kind="ExternalInput")
    io16_d = nc.dram_tensor("io16", [P, N_CLUSTERS], f16,
                            kind="ExternalInput")
    m8_d = nc.dram_tensor("m8", [R, 8], f32, kind="ExternalOutput")
    i8_d = nc.dram_tensor("i8", [R, 8], u32, kind="ExternalOutput")
    dw_d = nc.dram_tensor("dw", [N_CLUSTERS, EMBED_DIM], f32,
                          kind="ExternalOutput")

    with tile.TileContext(nc) as tc:
        with (
            tc.tile_pool(name="const", bufs=1) as const,
            tc.tile_pool(name="score", bufs=3) as spool,
            tc.tile_pool(name="small", bufs=4) as small,
            tc.tile_pool(name="oh", bufs=4) as ohpool,
            tc.tile_pool(name="ev", bufs=2) as evpool,
        ):
            xnt_sb = const.tile([P, DC, R], f16)
            ct_sb = const.tile([P, DC, N_CLUSTERS], f16)
            xb_sb = const.tile([P, RC, EMBED_DIM], f16)
            one2_sb = const.tile([P, P], f16)
            bias2_sb = const.tile([P, N_CLUSTERS], f16)
            io16_sb = const.tile([P, N_CLUSTERS], f16)
            idxf_sb = const.tile([P, RC], f32)
            m8all = const.tile([P, RC * 8], f32)
            i8all = const.tile([P, RC * 8], u32)

            # One dma_start = one DMA queue, so split every load per-dc and
            # issue in critical-path order: ct + bias + xnt piece0 feed
            # chunk 0; phase-B-only loads (xb, io16) go last.
            for dc in range(DC):
                nc.sync.dma_start(ct_sb[:, dc, :],
                                  ct_d[dc * P:(dc + 1) * P, :])
            nc.sync.dma_start(one2_sb[0:2, :], one2_d[:])
            nc.sync.dma_start(bias2_sb[0:2, :], bias2_d[:])
            NPIECE = 4
            W = R // NPIECE
            for pc in range(NPIECE):
                n0, n1 = pc * W, (pc + 1) * W
                for dc in range(DC):
                    nc.sync.dma_start(xnt_sb[:, dc, n0:n1],
                                      xnt_d[dc * P:(dc + 1) * P, n0:n1])
            xb_r = xb_d[:].rearrange("(i p) d -> p i d", p=P)
            for pc in range(NPIECE):
                i0, i1 = pc * (RC // NPIECE), (pc + 1) * (RC // NPIECE)
                nc.sync.dma_start(xb_sb[:, i0:i1, :], xb_r[:, i0:i1, :])
            nc.sync.dma_start(io16_sb[:], io16_d[:])

            # ---- Phase A: scores + argmax per row chunk ----
            with tc.tile_pool(name="psA", bufs=2,
                              space=bass.MemorySpace.PSUM) as psA:
                for i in range(RC):
                    score = spool.tile([P, N_CLUSTERS], f32, tag="score")
                    sw = psA.tile([P, N_CLUSTERS], f32, tag="ps")
                    for kc in range(KC):
                        s = sw[:, kc * 512:(kc + 1) * 512]
                        for dc in range(DC):
                            nc.tensor.matmul(
                                s,
                                xnt_sb[:, dc, i * P:(i + 1) * P],
                                ct_sb[:, dc, kc * 512:(kc + 1) * 512],
                                start=(dc == 0), stop=False)
                        nc.tensor.matmul(
                            s,
                            one2_sb[0:2, :],
                            bias2_sb[0:2, kc * 512:(kc + 1) * 512],
                            start=False, stop=True)
                    nc.scalar.copy(score[:], sw[:])
                    m8 = m8all[:, i * 8:(i + 1) * 8]
                    i8 = i8all[:, i * 8:(i + 1) * 8]
                    nc.vector.max(m8, score[:])
                    nc.vector.max_index(i8, m8, score[:])
                    nc.vector.tensor_copy(idxf_sb[:, i:i + 1], i8[:, 0:1])

            nc.sync.dma_start(m8_d[:].rearrange("(i p) j -> p i j", p=P),
                              m8all[:].rearrange("p (i j) -> p i j", j=8))
            nc.sync.dma_start(i8_d[:].rearrange("(i p) j -> p i j", p=P),
                              i8all[:].rearrange("p (i j) -> p i j", j=8))

            # ---- Phase B: dw = onehot^T @ X, 4 k-quarter passes so one
            # quarter's evacuation overlaps the next quarter's matmuls ----
            KQ = 4
            KOQ = KO * KH // KQ  # 4 psum banks per quarter
            with tc.tile_pool(name="psB", bufs=2,
                              space=bass.MemorySpace.PSUM) as psB:
                for q in range(KQ):
                    ps = [psB.tile([P, EMBED_DIM], f32, tag=f"dw{ko}",
                                   name=f"psdw_{q}_{ko}")
                          for ko in range(KOQ)]
                    for i in range(RC):
                        oh = ohpool.tile([P, KOQ * P], f16, tag="oh")
                        nc.vector.tensor_scalar(
                            out=oh[:],
                            in0=io16_sb[:, q * KOQ * P:(q + 1) * KOQ * P],
                            scalar1=idxf_sb[:, i:i + 1],
                            scalar2=None,
                            op0=mybir.AluOpType.is_equal)
                        for ko in range(KOQ):
                            nc.tensor.matmul(
                                ps[ko][:],
                                oh[:, ko * P:(ko + 1) * P],
                                xb_sb[:, i, :],
                                start=(i == 0), stop=(i == RC - 1))
                    for ko in range(KOQ):
                        ev = evpool.tile([P, EMBED_DIM], f32, tag="ev")
                        nc.scalar.copy(ev[:], ps[ko][:])
                        k0 = (q * KOQ + ko) * P
                        nc.sync.dma_start(dw_d[k0:k0 + P, :], ev[:])

    nc.compile()
    return nc


_NC_CACHE = None


def _get_nc():
    global _NC_CACHE
    if _NC_CACHE is None:
        _NC_CACHE = build_nc()
    return _NC_CACHE


def make_in_maps(X, centroids):
    norms = np.linalg.norm(X, axis=1, keepdims=True)
    Xn = X / np.maximum(norms, NORM_EPS)
    XhT = np.ascontiguousarray(Xn.astype(np.float16).T)    # [512, 32768]
    CT = np.ascontiguousarray(centroids.T.astype(np.float16))
    bias = -0.5 * (centroids * centroids).sum(axis=1)      # [2048]
    bh = bias.astype(np.float16)
    bl = ((bias - bh.astype(np.float32)) * 2048.0).astype(np.float16)
    bias2 = np.stack([bh, bl])                             # [2, 2048]
    one2 = np.empty((2, P), np.float16)
    one2[0] = 1.0
    one2[1] = 2.0 ** -11
    io16 = np.ascontiguousarray(np.broadcast_to(
        np.arange(N_CLUSTERS, dtype=np.float16)[None, :], (P, N_CLUSTERS)))
    xb16 = X.astype(np.float16)
    in_maps = []
    for c in range(N_CORES):
        sl = slice(c * R, (c + 1) * R)
        in_maps.append({
            "xnt": np.ascontiguousarray(XhT[:, sl]),
            "xb": np.ascontiguousarray(xb16[sl]),
            "ct": CT,
            "one2": one2,
            "bias2": bias2,
            "io16": io16,
        })
    return in_maps


def postprocess(X, centroids, ema_cluster_size, ema_w, idx_full, dw):
    counts = np.bincount(idx_full, minlength=N_CLUSTERS).astype(np.float32)
    quantized = centroids[idx_full]
    new_size = ema_cluster_size * DECAY + (1.0 - DECAY) * counts
    n = new_size.sum(dtype=np.float32)
    new_size = (new_size + EPSILON) / (n + N_CLUSTERS * EPSILON) * n
    new_w = ema_w * DECAY + (1.0 - DECAY) * dw
    new_centroids = new_w / new_size[:, None]
    return (quantized, idx_full[:, None].astype(np.int32), new_centroids,
            new_size, new_w)


def kernel(X, centroids, ema_cluster_size, ema_w):
    X = np.asarray(X, dtype=np.float32)
    centroids = np.asarray(centroids, dtype=np.float32)
    ema_cluster_size = np.asarray(ema_cluster_size, dtype=np.float32)
    ema_w = np.asarray(ema_w, dtype=np.float32)

    nc = _get_nc()
    in_maps = make_in_maps(X, centroids)
    res = None
    last_exc = None
    for attempt in range(4):
        try:
            res = run_bass_kernel_spmd(nc, in_maps, list(range(N_CORES)))
            break
        except Exception as e:  # transient device errors: reset + retry
            last_exc = e
            try:
                import ctypes
                lib = ctypes.CDLL('/opt/axon/libaxon_pjrt.so')
                lib.axon_reset.restype = ctypes.c_int64
                lib.axon_reset()
            except Exception:
                pass
            import time
            time.sleep(30 * (attempt + 1))
    if res is None:
        raise last_exc

    m8 = np.concatenate([res.results[c]["m8"] for c in range(N_CORES)])
    i8 = np.concatenate([res.results[c]["i8"] for c in range(N_CORES)])
    idx_full = i8[:, 0].astype(np.int32)
    dw = np.zeros((N_CLUSTERS, EMBED_DIM), dtype=np.float32)
    for c in range(N_CORES):
        dw += res.results[c]["dw"]

    # Low-margin repair: the device scores are fp16-operand matmuls
    # (error <~2e-3); rows whose top-2 gap is under 8e-3 get their top-8
    # candidates rescored exactly in fp32 on host, and idx/dw are patched
    # (dw is a linear sum, so a repair moves one X row between clusters).
    margin = m8[:, 0] - m8[:, 1]
    rows = np.where(margin < 8e-3)[0]
    if rows.size:
        Xn = X / np.maximum(np.linalg.norm(X, axis=1, keepdims=True),
                            NORM_EPS)
        c2 = (centroids * centroids).sum(axis=1)
        for r in rows:
            cand = np.unique(i8[r].astype(np.int64))
            cand = cand[(cand >= 0) & (cand < N_CLUSTERS)]
            sc = Xn[r] @ centroids[cand].T - 0.5 * c2[cand]
            k_new = int(cand[sc == sc.max()].min())
            k_old = int(idx_full[r])
            if k_new != k_old:
                dw[k_old] -= X[r]
                dw[k_new] += X[r]
                idx_full[r] = k_new
    return postprocess(X, centroids, ema_cluster_size, ema_w, idx_full, dw)
